# revision 1
# baseline (speedup 1.0000x reference)
"""Trainium2 Bass kernel for nn_DARPDecoder (sparse_attention).

Strategy (pure data-parallel over batch, 8 cores x 128 batches):
  score[b,n] = emb[b,n,:] . qk[b] / sqrt(D) - travel[b,n]*c ; tanh-clip, mask,
  log_softmax.  qk[b] = W_key^T q[b] eliminates the [B,N,D] K intermediate.
Per core, two bf16 streaming passes over the 16MB embedding shard:
  pass 1 (natural [n,d] tiles): graph/visited sums via accumulating matmuls
    with per-batch zero-padded [128,32] stationaries + tile_position, so each
    batch lands on its own PSUM rows.
  pass 2 (host-pre-transposed [d,n] tiles): per-batch score matmuls with
    zero-padded qk columns accumulate into one [128,512] PSUM tile (batch ->
    partition), giving the softmax layout for free.
Gathers: indirect DMA (rows); travel lookup T[cur_h3[b], h3[b,n]] via gpsimd
indirect_copy, 8 batches per call (one per 16-partition group), with host
pre-wrapped h3 index layout and a selection-matmul replicating each batch's
travel-time row across its group.
"""

import functools
import math

import numpy as np
import ml_dtypes

import concourse.bass as bass
import concourse.mybir as mybir
import concourse.tile as tile
from concourse.tile_rust import add_dep_helper
from concourse import bacc
from concourse.bass_utils import run_bass_kernel_spmd

BF16 = mybir.dt.bfloat16
F32 = mybir.dt.float32
I32 = mybir.dt.int32
U16 = mybir.dt.uint16
U8 = mybir.dt.uint8
Alu = mybir.AluOpType
AF = mybir.ActivationFunctionType
AX = mybir.AxisListType

B, N, D, NCORES = 1024, 512, 128, 8
BC = B // NCORES  # 128 batches/core
NCH, CHB = 16, 8  # 16 stream chunks x 8 batches
MAX_TIME = 1440.0
TANH_CLIP = 10.0
C_TRAVEL = 1.0 / MAX_TIME / math.sqrt(2.0)
INV_SQRT_D = 1.0 / math.sqrt(D)
NBF = np.dtype(ml_dtypes.bfloat16)
DEBUG_TAPS = False


def _emit(nc, tc, T):
    """Emit the whole per-core program. T: dict of dram tensor handles."""
    ap = {k: v.ap() for k, v in T.items()}

    with (
        tc.tile_pool(name="cp", bufs=1) as cp,
        tc.tile_pool(name="st", bufs=3) as st,
        tc.tile_pool(name="wk", bufs=2) as wk,
        tc.tile_pool(name="ps_sum", bufs=1, space="PSUM") as ps_sum,
        tc.tile_pool(name="ps_sm", bufs=1, space="PSUM") as ps_sm,
        tc.tile_pool(name="ps_tr", bufs=2, space="PSUM") as ps_tr,
        tc.tile_pool(name="ps_sc", bufs=1, space="PSUM") as ps_sc,
    ):
        def load(name, shape, dtype, src_ap=None, tag=None):
            t = cp.tile(shape, dtype, name=name, tag=tag or name)
            nc.sync.dma_start(out=t[:], in_=src_ap if src_ap is not None else ap[name])
            return t

        # ---------- small loads ----------
        wl = load("w_last", [D, D], BF16)
        wf = load("w_first", [D, D], BF16)
        wg = load("w_graph", [D, D], BF16)
        wv = load("w_visited", [D, D], BF16)
        wkT = load("w_keyT", [D, D], BF16)
        ws = load("w_state", [3, D], BF16)
        bst = load("b_state", [D, 1], F32)
        sc4 = load("scal4", [BC, 4], F32)
        cur = load("cur_i", [BC, 1], I32)
        prv = load("prev_i", [BC, 1], I32)
        fst = load("first_i", [BC, 1], I32)
        io5 = load("iota512f", [BC, 1], F32)
        idn = load("ident", [128, 128], BF16)
        pa = load("p_a", [128, 128], BF16)
        pb = load("p_b", [128, 128], BF16)
        chf = load("c_half", [128, 1], F32)
        vis_u8 = load("vis_rows", [BC, N], U8)
        am_u8 = load("am_rows", [BC, N], U8)
        ssel = [load(f"ssel{k}", [128, 128], BF16, ap["s_sel"][k]) for k in range(16)]
        h3w = [load(f"h3w{k}", [128, N // 16], U16, ap["h3_wrap"][k]) for k in range(16)]
        visT = [load(f"visT{t}", [128, BC], BF16, ap["visT_bf"][128 * t : 128 * (t + 1), :]) for t in range(4)]

        # ---------- masks / counts ----------
        visf = cp.tile([BC, N], F32, name="visf")
        nc.vector.tensor_copy(out=visf[:], in_=vis_u8[:])
        amf = cp.tile([BC, N], F32, name="amf")
        nc.vector.tensor_copy(out=amf[:], in_=am_u8[:])
        vc = cp.tile([BC, 1], F32, name="vc")
        nc.vector.tensor_reduce(out=vc[:], in_=visf[:], axis=AX.X, op=Alu.add)
        nc.vector.tensor_scalar_max(out=vc[:], in0=vc[:], scalar1=1.0)
        vcr = cp.tile([BC, 1], F32, name="vcr")
        nc.vector.reciprocal(out=vcr[:], in_=vc[:])
        vcrb = cp.tile([BC, 1], BF16, name="vcrb")
        nc.vector.tensor_copy(out=vcrb[:], in_=vcr[:])
        # per-PSUM-row descale vector: even rows 1/512 (graph mean), odd rows 1/vcount
        vcrp = []
        for half, P in ((0, pa), (1, pb)):
            pm = ps_sm.tile([128, 1], F32, tag="sm")
            nc.tensor.matmul(out=pm[:], lhsT=P[:], rhs=vcrb[:], start=True, stop=True)
            vp = cp.tile([128, 1], F32, name=f"vcrp{half}", tag=f"vcrp{half}")
            nc.vector.tensor_add(out=vp[:], in0=pm[:], in1=chf[:])
            vcrp.append(vp)

        # ---------- first-node bookkeeping + gather indices (f32 exact) ----------
        curf = cp.tile([BC, 1], F32, name="curf")
        nc.vector.tensor_copy(out=curf[:], in_=cur[:])
        prvf = cp.tile([BC, 1], F32, name="prvf")
        nc.vector.tensor_copy(out=prvf[:], in_=prv[:])
        fstf = cp.tile([BC, 1], F32, name="fstf")
        nc.vector.tensor_copy(out=fstf[:], in_=fst[:])
        t1 = cp.tile([BC, 1], F32, name="t1")
        nc.vector.tensor_single_scalar(out=t1[:], in_=prvf[:], scalar=0.0, op=Alu.is_equal)
        t2 = cp.tile([BC, 1], F32, name="t2")
        nc.vector.tensor_single_scalar(out=t2[:], in_=curf[:], scalar=0.0, op=Alu.not_equal)
        ld = cp.tile([BC, 1], F32, name="ld")
        nc.vector.tensor_mul(out=ld[:], in0=t1[:], in1=t2[:])
        dd = cp.tile([BC, 1], F32, name="dd")
        nc.vector.tensor_sub(out=dd[:], in0=curf[:], in1=fstf[:])
        nc.vector.tensor_mul(out=dd[:], in0=ld[:], in1=dd[:])
        fnf = cp.tile([BC, 1], F32, name="fnf")
        nc.vector.tensor_add(out=fnf[:], in0=fstf[:], in1=dd[:])
        nc.vector.tensor_mul(out=fnf[:], in0=fnf[:], in1=t2[:])

        gcf = cp.tile([BC, 1], F32, name="gcf")
        nc.vector.tensor_add(out=gcf[:], in0=io5[:], in1=curf[:])
        gcur = cp.tile([BC, 1], I32, name="gcur")
        nc.vector.tensor_copy(out=gcur[:], in_=gcf[:])
        gff = cp.tile([BC, 1], F32, name="gff")
        nc.vector.tensor_add(out=gff[:], in0=io5[:], in1=fnf[:])
        gfn = cp.tile([BC, 1], I32, name="gfn")
        nc.vector.tensor_copy(out=gfn[:], in_=gff[:])

        # ---------- gathers ----------
        hc_rows = cp.tile([BC, D], BF16, name="hc_rows")
        nc.gpsimd.indirect_dma_start(
            out=hc_rows[:], out_offset=None, in_=ap["emb_flat"],
            in_offset=bass.IndirectOffsetOnAxis(ap=gcur[:, :1], axis=0))
        hf_rows = cp.tile([BC, D], BF16, name="hf_rows")
        nc.gpsimd.indirect_dma_start(
            out=hf_rows[:], out_offset=None, in_=ap["emb_flat"],
            in_offset=bass.IndirectOffsetOnAxis(ap=gfn[:, :1], axis=0))
        ch3 = cp.tile([BC, 1], I32, name="ch3")
        nc.gpsimd.indirect_dma_start(
            out=ch3[:], out_offset=None, in_=ap["h3_flat"],
            in_offset=bass.IndirectOffsetOnAxis(ap=gcur[:, :1], axis=0))
        rrow = cp.tile([BC, N], F32, name="rrow")
        nc.gpsimd.indirect_dma_start(
            out=rrow[:], out_offset=None, in_=ap["ttm"],
            in_offset=bass.IndirectOffsetOnAxis(ap=ch3[:, :1], axis=0))
        rbf = cp.tile([BC, N], BF16, name="rbf")
        nc.vector.tensor_copy(out=rbf[:], in_=rrow[:])

        # ---------- travel: T[cur_h3[b], h3[b,:]] * C_TRAVEL, 8 batches/call ----------
        trav = cp.tile([BC, N], F32, name="trav")
        for k in range(16):
            pt = ps_tr.tile([128, N], F32, tag="trep")
            nc.tensor.matmul(out=pt[:], lhsT=ssel[k][:], rhs=rbf[:], start=True, stop=True)
            sck = wk.tile([128, N], F32, tag="travsc")
            nc.vector.tensor_copy(out=sck[:], in_=pt[:])
            gk = wk.tile([128, N], F32, tag="travg")
            nc.gpsimd.indirect_copy(out=gk[:], data=sck[:], idxs=h3w[k][:],
                                    i_know_ap_gather_is_preferred=True)
            for g in range(8):  # extract batch row 16g+k (partition strides are
                r0 = 16 * g + k  # not AP-expressible; 8 tiny row DMAs instead)
                nc.sync.dma_start(out=trav[r0 : r0 + 1, :], in_=gk[r0 : r0 + 1, :])

        # ---------- W2 stationaries for the sums pass ----------
        # per n-tile t: [128, 4096]; batch b owns cols [32b, 32b+32):
        #   col 32b+2s   = 1.0   (s = b%16)  -> graph sum row
        #   col 32b+2s+1 = vf_b  -> visited sum row
        w2 = []
        for t in range(4):
            w = cp.tile([128, 4096], BF16, name=f"w2_{t}", tag=f"w2_{t}")
            nc.vector.memset(w[:], 0.0)
            w3 = w[:].rearrange("p (u s) -> p u s", u=8)  # u: 512-col blocks, s: 34-col stride pairs
            ones_ap = w[:].rearrange("p (u c) -> p u c", u=8)[:, :, 0:512:34]
            nc.vector.memset(ones_ap, 1.0)
            vf_ap = w[:].rearrange("p (u c) -> p u c", u=8)[:, :, 1:512:34]
            nc.vector.tensor_copy(out=vf_ap, in_=visT[t][:].rearrange("p (u s) -> p u s", u=8))
            w2.append(w)

        # ---------- pass 1: natural-layout stream -> graph/visited sums ----------
        psA = ps_sum.tile([128, D], F32, tag="sumA")
        psB = ps_sum.tile([128, D], F32, tag="sumB")
        for k in range(NCH):
            nat = st.tile([128, 4096], BF16, tag="nat")
            nc.sync.dma_start(out=nat[:], in_=ap["emb_nat_t"][k])
            for j in range(CHB):
                b = k * CHB + j
                half, r = b // 64, b % 64
                jj, s = r // 16, r % 16
                ps = psA if half == 0 else psB
                for t in range(4):
                    nc.tensor.matmul(
                        out=ps[32 * jj : 32 * jj + 32, :],
                        lhsT=w2[t][:, 32 * b : 32 * b + 32],
                        rhs=nat[:, (j * 4 + t) * 128 : (j * 4 + t + 1) * 128],
                        start=(s == 0 and t == 0), stop=True,
                        tile_position=(0, 32 * jj), skip_group_check=True)

        # ---------- sums -> G^T / V^T (dense, bf16, [128e, 128b]) ----------
        gt = cp.tile([128, BC], BF16, name="gt")
        vt = cp.tile([128, BC], BF16, name="vt")
        for half, ps in ((0, psA), (1, psB)):
            gvr = wk.tile([128, 128], BF16, tag="gvr")
            nc.vector.tensor_scalar(out=gvr[:], in0=ps[:], scalar1=vcrp[half][:, :1],
                                    scalar2=None, op0=Alu.mult)
            pt = ps_tr.tile([128, 128], BF16, tag="gvt", bufs=1)
            nc.tensor.transpose(out=pt[:], in_=gvr[:], identity=idn[:])
            # cols m=32j+2s -> batch 64*half+16j+s
            src_g = pt[:].rearrange("p (j c) -> p j c", j=4)[:, :, 0:32:2]
            src_v = pt[:].rearrange("p (j c) -> p j c", j=4)[:, :, 1:32:2]
            dst_g = gt[:, 64 * half : 64 * half + 64].rearrange("p (j s) -> p j s", j=4)
            dst_v = vt[:, 64 * half : 64 * half + 64].rearrange("p (j s) -> p j s", j=4)
            nc.vector.tensor_copy(out=dst_g, in_=src_g)
            nc.vector.tensor_copy(out=dst_v, in_=src_v)

        # ---------- h_cur/h_first transposes ----------
        hct = cp.tile([128, BC], BF16, name="hct")
        pt1 = ps_tr.tile([128, 128], BF16, tag="gvt", bufs=1)
        nc.tensor.transpose(out=pt1[:], in_=hc_rows[:], identity=idn[:])
        nc.vector.tensor_copy(out=hct[:], in_=pt1[:])
        hft = cp.tile([128, BC], BF16, name="hft")
        pt2 = ps_tr.tile([128, 128], BF16, tag="gvt", bufs=1)
        nc.tensor.transpose(out=pt2[:], in_=hf_rows[:], identity=idn[:])
        nc.vector.tensor_copy(out=hft[:], in_=pt2[:])

        # ---------- state feats ----------
        sf = cp.tile([BC, 3], F32, name="sf")
        nc.vector.tensor_sub(out=sf[:, 0:1], in0=sc4[:, 2:3], in1=sc4[:, 1:2])
        nc.vector.tensor_scalar_mul(out=sf[:, 1:2], in0=sc4[:, 0:1], scalar1=1.0 / MAX_TIME)
        nc.vector.tensor_scalar_mul(out=sf[:, 2:3], in0=sc4[:, 3:4], scalar1=1.0 / (2.0 * N))
        sfb = cp.tile([BC, 3], BF16, name="sfb")
        nc.vector.tensor_copy(out=sfb[:], in_=sf[:])
        pt3 = ps_tr.tile([128, 128], BF16, tag="gvt", bufs=1)
        nc.tensor.transpose(out=pt3[:3, :], in_=sfb[:], identity=idn[:])
        sft = cp.tile([3, BC], BF16, name="sft")
        nc.vector.tensor_copy(out=sft[:], in_=pt3[:3, :BC])

        # ---------- q^T and qk^T ----------
        psq = ps_sm.tile([128, BC], F32, tag="sm")
        nc.tensor.matmul(out=psq[:], lhsT=wl[:], rhs=hct[:], start=True, stop=True)
        nc.tensor.matmul(out=psq[:], lhsT=wf[:], rhs=hft[:], start=False, stop=True,
                         skip_group_check=True)
        nc.tensor.matmul(out=psq[:], lhsT=wg[:], rhs=gt[:], start=False, stop=True,
                         skip_group_check=True)
        nc.tensor.matmul(out=psq[:], lhsT=wv[:], rhs=vt[:], start=False, stop=True,
                         skip_group_check=True)
        nc.tensor.matmul(out=psq[:], lhsT=ws[:], rhs=sft[:], start=False, stop=True,
                         skip_group_check=True)
        qt = cp.tile([128, BC], BF16, name="qt")
        nc.vector.tensor_scalar(out=qt[:], in0=psq[:], scalar1=bst[:, :1], scalar2=None,
                                op0=Alu.add)
        psk = ps_sm.tile([128, BC], F32, tag="sm")
        nc.tensor.matmul(out=psk[:], lhsT=wkT[:], rhs=qt[:], start=True, stop=True)
        qk = cp.tile([128, BC], BF16, name="qk")
        nc.vector.tensor_scalar_mul(out=qk[:], in0=psk[:], scalar1=INV_SQRT_D)

        # ---------- qkpad: batch b -> col 32b + (b%32) ----------
        qkp = cp.tile([128, 4096], BF16, name="qkp")
        nc.vector.memset(qkp[:], 0.0)
        for J in range(4):
            # batch b = 32J + r -> col 32b + r = 1024J + 33r (out row = 32J + r = b)
            nc.vector.tensor_copy(out=qkp[:, 1024 * J : 1024 * (J + 1) : 33],
                                  in_=qk[:, 32 * J : 32 * (J + 1)])

        # ---------- pass 2: transposed stream -> score psum [128b, 512n] ----------
        pssc = ps_sc.tile([128, N], F32, tag="score")
        for k in range(NCH):
            et = st.tile([128, 4096], BF16, tag="et")
            nc.sync.dma_start(out=et[:], in_=ap["emb_T_t"][k])
            for j in range(CHB):
                b = k * CHB + j
                J, r = b // 32, b % 32
                nc.tensor.matmul(
                    out=pssc[32 * J : 32 * J + 32, :],
                    lhsT=qkp[:, 32 * b : 32 * b + 32],
                    rhs=et[:, j * N : (j + 1) * N],
                    start=(r == 0), stop=True,
                    tile_position=(0, 32 * J), skip_group_check=True)

        # ---------- epilogue: travel, tanh, mask, log_softmax ----------
        if DEBUG_TAPS:
            for nm, tl in [("d_trav", trav), ("d_gt", gt), ("d_vt", vt),
                           ("d_hct", hct), ("d_hft", hft), ("d_qt", qt), ("d_qk", qk)]:
                tmpd = wk.tile([tl.shape[0], tl.shape[1]], F32, tag=f"tap{nm}")
                nc.vector.tensor_copy(out=tmpd[:], in_=tl[:])
                nc.sync.dma_start(out=ap[nm], in_=tmpd[:])
        ssb = wk.tile([BC, N], F32, tag="ssb")
        nc.vector.tensor_sub(out=ssb[:], in0=pssc[:], in1=trav[:])
        if DEBUG_TAPS:
            nc.sync.dma_start(out=ap["d_score"], in_=ssb[:])
        th = wk.tile([BC, N], F32, tag="th")
        nc.scalar.activation(out=th[:], in_=ssb[:], func=AF.Tanh, scale=1.0 / TANH_CLIP)
        m10 = wk.tile([BC, N], F32, tag="m10")
        nc.vector.tensor_scalar_mul(out=m10[:], in0=amf[:], scalar1=TANH_CLIP)
        m2 = wk.tile([BC, N], F32, tag="m2")
        nc.vector.tensor_scalar(out=m2[:], in0=amf[:], scalar1=1.0, scalar2=1e8,
                                op0=Alu.subtract, op1=Alu.mult)
        msk = wk.tile([BC, N], F32, tag="msk")
        nc.vector.tensor_mul(out=msk[:], in0=th[:], in1=m10[:])
        nc.vector.tensor_add(out=msk[:], in0=msk[:], in1=m2[:])
        if DEBUG_TAPS:
            nc.sync.dma_start(out=ap["d_msk"], in_=msk[:])

        mx = cp.tile([BC, 1], F32, name="mx")
        nc.vector.tensor_reduce(out=mx[:], in_=msk[:], axis=AX.X, op=Alu.max)
        ngm = cp.tile([BC, 1], F32, name="ngm")
        nc.vector.tensor_scalar_mul(out=ngm[:], in0=mx[:], scalar1=-1.0)
        ex = wk.tile([BC, N], F32, tag="ex")
        se = cp.tile([BC, 1], F32, name="se")
        nc.scalar.activation(out=ex[:], in_=msk[:], func=AF.Exp, bias=ngm[:, :1],
                             scale=1.0, accum_out=se[:])
        lse = cp.tile([BC, 1], F32, name="lse")
        nc.scalar.activation(out=lse[:], in_=se[:], func=AF.Ln)
        sh = wk.tile([BC, N], F32, tag="sh")
        nc.vector.tensor_scalar(out=sh[:], in0=msk[:], scalar1=mx[:, :1], scalar2=None,
                                op0=Alu.subtract)
        fin = wk.tile([BC, N], F32, tag="fin")
        nc.vector.tensor_scalar(out=fin[:], in0=sh[:], scalar1=lse[:, :1], scalar2=None,
                                op0=Alu.subtract)
        nc.sync.dma_start(out=ap["out"], in_=fin[:])


def build_program():
    nc = bacc.Bacc("TRN2", target_bir_lowering=False, debug=False)
    dt = nc.dram_tensor
    T = {}

    def din(name, shape, dtype):
        T[name] = dt(name, shape, dtype, kind="ExternalInput")

    din("emb_nat_t", [NCH, 128, CHB * N * D // 128], BF16)
    din("emb_T_t", [NCH, 128, CHB * N], BF16)
    din("emb_flat", [BC * N, D], BF16)
    din("h3_flat", [BC * N, 1], I32)
    din("h3_wrap", [NCH, 128, N // 16], U16)
    din("ttm", [N, N], F32)
    din("vis_rows", [BC, N], U8)
    din("visT_bf", [N, BC], BF16)
    din("am_rows", [BC, N], U8)
    for w in ("w_last", "w_first", "w_graph", "w_visited", "w_keyT"):
        din(w, [D, D], BF16)
    din("w_state", [3, D], BF16)
    din("b_state", [D, 1], F32)
    din("scal4", [BC, 4], F32)
    din("cur_i", [BC, 1], I32)
    din("prev_i", [BC, 1], I32)
    din("first_i", [BC, 1], I32)
    din("iota512f", [BC, 1], F32)
    din("ident", [128, 128], BF16)
    din("s_sel", [NCH, 128, 128], BF16)
    din("p_a", [128, 128], BF16)
    din("p_b", [128, 128], BF16)
    din("c_half", [128, 1], F32)
    T["out"] = dt("out", [BC, N], F32, kind="ExternalOutput")
    if DEBUG_TAPS:
        for nm, shp in [("d_trav", [BC, N]), ("d_gt", [128, BC]), ("d_vt", [128, BC]),
                        ("d_hct", [128, BC]), ("d_hft", [128, BC]), ("d_qt", [128, BC]),
                        ("d_qk", [128, BC]), ("d_score", [BC, N]), ("d_msk", [BC, N])]:
            T[nm] = dt(nm, shp, F32, kind="ExternalOutput")

    with tile.TileContext(nc) as tc:
        _emit(nc, tc, T)
    nc.compile()
    return nc


@functools.cache
def _cached_program():
    return build_program()


def _consts():
    c = {}
    c["ident"] = np.eye(128, dtype=NBF)
    s = np.zeros((16, 128, 128), dtype=NBF)
    pidx = np.arange(128)
    for k in range(16):
        s[k, (pidx // 16) * 16 + k, pidx] = np.float32(C_TRAVEL)
    c["s_sel"] = s
    pa = np.zeros((128, 128), dtype=NBF)
    pb = np.zeros((128, 128), dtype=NBF)
    for b in range(64):
        m = 32 * (b // 16) + 2 * (b % 16) + 1
        pa[b, m] = 1
        pb[64 + b, m] = 1
    c["p_a"], c["p_b"] = pa, pb
    ch = np.zeros((128, 1), np.float32)
    ch[0::2] = 1.0 / N
    c["c_half"] = ch
    c["iota512f"] = (np.arange(BC, dtype=np.float32) * N)[:, None]
    return c


def make_in_map(inputs, core, consts=None):
    """Host-side shard + relayout for one core (pure layout/dtype work)."""
    sl = slice(BC * core, BC * (core + 1))
    emb = np.asarray(inputs["node_emb"][sl], dtype=np.float32)
    embb = emb.astype(NBF)  # [128, 512, 128]
    m = {}
    m["emb_nat_t"] = np.ascontiguousarray(
        embb.reshape(NCH, CHB, 4, 128, D).transpose(0, 3, 1, 2, 4)).reshape(NCH, 128, CHB * 4 * D)
    embT = np.ascontiguousarray(embb.transpose(0, 2, 1))  # [128b, 128d, 512n]
    m["emb_T_t"] = np.ascontiguousarray(
        embT.reshape(NCH, CHB, 128, N).transpose(0, 2, 1, 3)).reshape(NCH, 128, CHB * N)
    m["emb_flat"] = embb.reshape(BC * N, D)
    h3 = np.asarray(inputs["h3_indices"][sl]).astype(np.int32)  # [128, 512]
    m["h3_flat"] = h3.reshape(BC * N, 1)
    m["h3_wrap"] = np.ascontiguousarray(
        h3.reshape(8, 16, 32, 16).transpose(1, 0, 3, 2)).reshape(16, 128, 32).astype(np.uint16)
    m["ttm"] = np.asarray(inputs["travel_time_matrix"], dtype=np.float32)
    vis = np.asarray(inputs["visited"][sl]).astype(np.uint8)
    m["vis_rows"] = vis
    m["visT_bf"] = np.ascontiguousarray(vis.T).astype(NBF)
    m["am_rows"] = np.asarray(inputs["action_mask"][sl]).astype(np.uint8)
    m["w_last"] = np.asarray(inputs["W_last"], np.float32).astype(NBF)
    m["w_first"] = np.asarray(inputs["W_first"], np.float32).astype(NBF)
    m["w_graph"] = np.asarray(inputs["W_graph"], np.float32).astype(NBF)
    m["w_visited"] = np.asarray(inputs["W_visited"], np.float32).astype(NBF)
    m["w_keyT"] = np.ascontiguousarray(np.asarray(inputs["W_key"], np.float32).T).astype(NBF)
    m["w_state"] = np.asarray(inputs["W_state"], np.float32).astype(NBF)
    m["b_state"] = np.asarray(inputs["b_state"], np.float32).reshape(D, 1)
    m["scal4"] = np.ascontiguousarray(np.concatenate(
        [np.asarray(inputs["current_time"][sl], np.float32),
         np.asarray(inputs["used_capacity"][sl], np.float32),
         np.asarray(inputs["vehicle_capacity"][sl], np.float32),
         np.asarray(inputs["i"][sl]).astype(np.float32)], axis=1))
    m["cur_i"] = np.asarray(inputs["current_node"][sl]).astype(np.int32).reshape(BC, 1)
    m["prev_i"] = np.asarray(inputs["previous_action"][sl]).astype(np.int32).reshape(BC, 1)
    m["first_i"] = np.asarray(inputs["first_node"][sl]).astype(np.int32).reshape(BC, 1)
    m.update(consts or _consts())
    return m


_last_results = None


def kernel(**inputs):
    global _last_results
    nc = _cached_program()
    consts = _consts()
    in_maps = [make_in_map(inputs, c, consts) for c in range(NCORES)]
    import os
    trace = bool(int(os.environ.get("KERNEL_TRACE", "0")))
    rr = run_bass_kernel_spmd(nc, in_maps, list(range(NCORES)), trace=trace)
    _last_results = rr
    out = np.concatenate([np.asarray(rr.results[c]["out"], np.float32)
                          for c in range(NCORES)], axis=0)
    return out



# revision 9
# speedup vs baseline: 1.1964x; 1.1964x over previous
"""Trainium2 Bass kernel for nn_DARPDecoder (sparse_attention).

Strategy (pure data-parallel over batch, 8 cores x 128 batches), single-pass
per-group pipeline:
  score[b,n] = emb[b,n,:].qk[b]/sqrt(D) - travel[b,n]*c ; tanh-clip, mask,
  log_softmax, with qk[b] = W_key^T q[b].
Per core, TWO HBM streams consumed in 8 groups of 16 batches each:
  - nat_f8 (natural [n,d] tiles, fp8-e4m3, 8.4MB): graph/visited sums via
    "flipped" matmuls -- the nat tile is the STATIONARY, the moving operand is
    a tiny [128,2] (const 0.25 | 0.25*vf) column pair, accumulating [128d, 2]
    per batch into a per-group PSUM [128, 32].  fp8 is plenty for sums.
  - et_bf (transposed [d,n] tiles, bf16, 16.8MB): per-batch score matmuls with
    zero-padded qk columns accumulate into one [128b, 512n] PSUM.
Each group's sums -> qk -> scores chases its chunk DMAs, so everything hides
behind the ~70us of HBM traffic.  Total DMA count is ~30 (vs 200+): travel
rows are extracted from the gpsimd gather output with 16 selection matmuls
accumulating into a PSUM travel tile instead of 128 tiny row DMAs.
Gathers: indirect DMA rows (h_cur/h_first from bf16 emb_flat, T rows by
cur_h3); travel lookup T[cur_h3[b], h3[b,n]] via gpsimd indirect_copy on
16-partition-replicated rows (host pre-wrapped h3 index layout).
Per-batch 1/vcount descale of the visited sum is applied exactly via a
transpose sandwich: V_raw^T -> [16b,128d] (batch on partitions) -> per-
partition tensor_scalar multiply -> transpose back.
"""

import functools
import math

import numpy as np
import ml_dtypes

import concourse.bass as bass
import concourse.mybir as mybir
import concourse.tile as tile
from concourse import bacc
from concourse.bass_utils import run_bass_kernel_spmd

BF16 = mybir.dt.bfloat16
F8 = mybir.dt.float8e4
F32 = mybir.dt.float32
I32 = mybir.dt.int32
U16 = mybir.dt.uint16
U8 = mybir.dt.uint8
Alu = mybir.AluOpType
AF = mybir.ActivationFunctionType
AX = mybir.AxisListType

B, N, D, NCORES = 1024, 512, 128, 8
BC = B // NCORES   # 128 batches/core
NG, GB = 8, 16     # 8 groups x 16 batches
MAX_TIME = 1440.0
TANH_CLIP = 10.0
C_TRAVEL = 1.0 / MAX_TIME / math.sqrt(2.0)
INV_SQRT_D = 1.0 / math.sqrt(D)
NBF = np.dtype(ml_dtypes.bfloat16)
NF8 = np.dtype(ml_dtypes.float8_e4m3)
DEBUG_TAPS = False


def _emit(nc, tc, T):
    ap = {k: v.ap() for k, v in T.items()}

    with (
        tc.tile_pool(name="cp", bufs=1) as cp,
        tc.tile_pool(name="wk", bufs=2) as wk,
        tc.tile_pool(name="stn", bufs=3) as stn,
        tc.tile_pool(name="ste", bufs=3) as ste,
        tc.tile_pool(name="ps_sc", bufs=1, space="PSUM") as ps_sc,
        tc.tile_pool(name="ps_tv", bufs=1, space="PSUM") as ps_tv,
        tc.tile_pool(name="ps_rep", bufs=2, space="PSUM") as ps_rep,
        tc.tile_pool(name="ps_sm", bufs=2, space="PSUM") as ps_sm,
        tc.tile_pool(name="ps_q", bufs=2, space="PSUM") as ps_q,
    ):
        def load(name, shape, dtype):
            t = cp.tile(shape, dtype, name=name, tag=name)
            nc.sync.dma_start(out=t[:], in_=ap[name])
            return t

        # ---- stream DMA issue: nat chunks first (sums/qk warm up), then et ----
        nat = [None] * NG
        et = [None] * NG
        nat[0] = stn.tile([128, GB * 4 * D], F8, tag="nat", name="nat0")
        nc.sync.dma_start(out=nat[0][:], in_=ap["nat_f8"][0])

        sc8 = load("sc8", [BC, 8], F32)
        iota = load("iota", [BC, 1], F32)
        vfc = load("vfc", [128, NG * 128], F8)
        for g in range(1, NG):
            nat[g] = stn.tile([128, GB * 4 * D], F8, tag="nat", name=f"nat{g}")
            nc.sync.dma_start(out=nat[g][:], in_=ap["nat_f8"][g])
        wcat = load("wcat", [D, 5 * D], BF16)
        ws = load("ws", [3, D], BF16)
        bst = load("bst", [D, 1], F32)
        visam = load("visam", [BC, 2 * N], U8)
        idn = load("ident", [128, 128], BF16)
        h3w = load("h3w", [128, 16 * 32], U16)
        sselc = load("sselc", [128, 16 * 128], BF16)
        dkc = load("dkc", [128, 16 * 128], BF16)
        for g in range(NG):
            et[g] = ste.tile([128, GB * N], BF16, tag="et", name=f"et{g}")
            nc.sync.dma_start(out=et[g][:], in_=ap["et_bf"][g])

        # ---- scalar state / first-node bookkeeping (DVE, f32 exact) ----
        t1 = cp.tile([BC, 1], F32, name="t1")
        nc.vector.tensor_single_scalar(out=t1[:], in_=sc8[:, 5:6], scalar=0.0,
                                       op=Alu.is_equal)
        t2 = cp.tile([BC, 1], F32, name="t2")
        nc.vector.tensor_single_scalar(out=t2[:], in_=sc8[:, 4:5], scalar=0.0,
                                       op=Alu.not_equal)
        ld = cp.tile([BC, 1], F32, name="ld")
        nc.vector.tensor_mul(out=ld[:], in0=t1[:], in1=t2[:])
        dd = cp.tile([BC, 1], F32, name="dd")
        nc.vector.tensor_sub(out=dd[:], in0=sc8[:, 4:5], in1=sc8[:, 6:7])
        nc.vector.tensor_mul(out=dd[:], in0=ld[:], in1=dd[:])
        fnf = cp.tile([BC, 1], F32, name="fnf")
        nc.vector.tensor_add(out=fnf[:], in0=sc8[:, 6:7], in1=dd[:])
        nc.vector.tensor_mul(out=fnf[:], in0=fnf[:], in1=t2[:])

        gcf = cp.tile([BC, 1], F32, name="gcf")
        nc.vector.tensor_add(out=gcf[:], in0=iota[:], in1=sc8[:, 4:5])
        gcur = cp.tile([BC, 1], I32, name="gcur")
        nc.vector.tensor_copy(out=gcur[:], in_=gcf[:])
        gff = cp.tile([BC, 1], F32, name="gff")
        nc.vector.tensor_add(out=gff[:], in0=iota[:], in1=fnf[:])
        gfn = cp.tile([BC, 1], I32, name="gfn")
        nc.vector.tensor_copy(out=gfn[:], in_=gff[:])

        # visited count -> vcr4 = 4/max(vc,1) (bf16 col for ident matmul rhs)
        visf = cp.tile([BC, N], F32, name="visf")
        nc.vector.tensor_copy(out=visf[:], in_=visam[:, :N])
        vc = cp.tile([BC, 1], F32, name="vc")
        nc.vector.tensor_reduce(out=vc[:], in_=visf[:], axis=AX.X, op=Alu.add)
        nc.vector.tensor_scalar_max(out=vc[:], in0=vc[:], scalar1=1.0)
        nc.vector.tensor_scalar_mul(out=vc[:], in0=vc[:], scalar1=0.25)
        vcr4 = cp.tile([BC, 1], F32, name="vcr4")
        nc.vector.reciprocal(out=vcr4[:], in_=vc[:])
        vcr4b = cp.tile([BC, 1], BF16, name="vcr4b")
        nc.vector.tensor_copy(out=vcr4b[:], in_=vcr4[:])

        # action mask precompute
        amf = cp.tile([BC, N], F32, name="amf")
        nc.vector.tensor_copy(out=amf[:], in_=visam[:, N:])
        m10 = cp.tile([BC, N], F32, name="m10")
        nc.vector.tensor_scalar_mul(out=m10[:], in0=amf[:], scalar1=TANH_CLIP)
        m2 = cp.tile([BC, N], F32, name="m2")
        nc.vector.tensor_scalar(out=m2[:], in0=amf[:], scalar1=1.0, scalar2=1e8,
                                op0=Alu.subtract, op1=Alu.mult)

        # state feats [BC,3] -> sft [3,BC]
        sfb = cp.tile([BC, 3], BF16, name="sfb")
        nc.vector.tensor_sub(out=sfb[:, 0:1], in0=sc8[:, 2:3], in1=sc8[:, 1:2])
        nc.vector.tensor_scalar_mul(out=sfb[:, 1:2], in0=sc8[:, 0:1],
                                    scalar1=1.0 / MAX_TIME)
        nc.vector.tensor_scalar_mul(out=sfb[:, 2:3], in0=sc8[:, 3:4],
                                    scalar1=1.0 / (2.0 * N))
        psf = ps_q.tile([128, 128], BF16, tag="sm")
        nc.tensor.transpose(out=psf[:3, :], in_=sfb[:], identity=idn[:])
        sft = cp.tile([3, BC], BF16, name="sft")
        nc.vector.tensor_copy(out=sft[:], in_=psf[:3, :BC])

        # wg scaled by 1/128 (sums use 0.25 weights; graph mean needs /512)
        wgs = cp.tile([D, D], BF16, name="wgs")
        nc.vector.tensor_scalar_mul(out=wgs[:], in0=wcat[:, 2 * D:3 * D],
                                    scalar1=1.0 / 128.0)

        # ---- gathers (Pool/SWDGE queue) ----
        hc_rows = cp.tile([BC, D], BF16, name="hc_rows")
        nc.gpsimd.indirect_dma_start(
            out=hc_rows[:], out_offset=None, in_=ap["emb_flat"],
            in_offset=bass.IndirectOffsetOnAxis(ap=gcur[:, :1], axis=0))
        hf_rows = cp.tile([BC, D], BF16, name="hf_rows")
        nc.gpsimd.indirect_dma_start(
            out=hf_rows[:], out_offset=None, in_=ap["emb_flat"],
            in_offset=bass.IndirectOffsetOnAxis(ap=gfn[:, :1], axis=0))
        ch3 = cp.tile([BC, 1], I32, name="ch3")
        nc.gpsimd.indirect_dma_start(
            out=ch3[:], out_offset=None, in_=ap["h3_flat"],
            in_offset=bass.IndirectOffsetOnAxis(ap=gcur[:, :1], axis=0))
        rrow = cp.tile([BC, N], BF16, name="rrow")
        nc.gpsimd.indirect_dma_start(
            out=rrow[:], out_offset=None, in_=ap["ttm_bf"],
            in_offset=bass.IndirectOffsetOnAxis(ap=ch3[:, :1], axis=0))

        # ---- h_cur / h_first transposes -> [128d, BC] bf16 ----
        hct = cp.tile([D, BC], BF16, name="hct")
        pt1 = ps_q.tile([128, 128], BF16, tag="sm")
        nc.tensor.transpose(out=pt1[:], in_=hc_rows[:], identity=idn[:])
        nc.vector.tensor_copy(out=hct[:], in_=pt1[:])
        hft = cp.tile([D, BC], BF16, name="hft")
        pt2 = ps_q.tile([128, 128], BF16, tag="sm")
        nc.tensor.transpose(out=pt2[:], in_=hf_rows[:], identity=idn[:])
        nc.vector.tensor_copy(out=hft[:], in_=pt2[:])

        # ---- travel: replicate rows, gpsimd gather, select-accumulate ----
        gk_all = cp.tile([128, 16 * N], BF16, name="gk_all")
        for k in range(16):
            prep = ps_rep.tile([128, N], F32, tag="rep")
            nc.tensor.matmul(out=prep[:], lhsT=sselc[:, 128 * k:128 * (k + 1)],
                             rhs=rrow[:], start=True, stop=True)
            sck = wk.tile([128, N], BF16, tag="sck")
            nc.scalar.activation(out=sck[:], in_=prep[:], func=AF.Copy)
            nc.gpsimd.indirect_copy(out=gk_all[:, N * k:N * (k + 1)],
                                    data=sck[:], idxs=h3w[:, 32 * k:32 * (k + 1)],
                                    i_know_ap_gather_is_preferred=True)

        # ---- qkp (zero-padded qk columns for the score matmuls) ----
        qkp = cp.tile([128, 32 * BC], BF16, name="qkp")
        nc.vector.memset(qkp[:], 0.0)

        pssc = ps_sc.tile([128, N], F32, tag="score")
        ptv = ps_tv.tile([128, N], F32, tag="trav")

        # ---- per-group pipeline: sums -> qk -> scores ----
        for g in range(NG):
            # sums: flipped matmuls, nat tile slices as stationaries
            psS = ps_sm.tile([128, 2 * GB], F32, tag="sums")
            for j in range(GB):
                for t in range(4):
                    nc.tensor.matmul(
                        out=psS[:, 2 * j:2 * j + 2],
                        lhsT=nat[g][:, (j * 4 + t) * D:(j * 4 + t + 1) * D],
                        rhs=vfc[:, 128 * g + 32 * t + 2 * j:
                                 128 * g + 32 * t + 2 * j + 2],
                        start=(t == 0), stop=(t == 3), skip_group_check=True)

            # graph cols (even) / raw visited cols (odd) -> SBUF bf16
            gt_g = wk.tile([D, GB], BF16, tag="gt")
            nc.vector.tensor_copy(
                out=gt_g[:], in_=psS[:].rearrange("p (s c) -> p s c", c=2)[:, :, 0])
            vr_g = wk.tile([D, GB], BF16, tag="vr")
            nc.vector.tensor_copy(
                out=vr_g[:], in_=psS[:].rearrange("p (s c) -> p s c", c=2)[:, :, 1])

            # 1/vcount descale sandwich: transpose, per-partition scale, back
            vcg = ps_q.tile([GB, 1], F32, tag="sm")
            nc.tensor.matmul(out=vcg[:], lhsT=idn[:, GB * g:GB * (g + 1)],
                             rhs=vcr4b[:], start=True, stop=True)
            vcgs = wk.tile([GB, 1], F32, tag="vcgs")
            nc.vector.tensor_copy(out=vcgs[:], in_=vcg[:])
            pvt = ps_q.tile([GB, D], BF16, tag="sm")
            nc.tensor.transpose(out=pvt[:], in_=vr_g[:], identity=idn[:])
            vts = wk.tile([GB, D], BF16, tag="vts")
            nc.vector.tensor_scalar(out=vts[:], in0=pvt[:], scalar1=vcgs[:, :1],
                                    scalar2=None, op0=Alu.mult)
            pvb = ps_q.tile([D, GB], F32, tag="sm")
            nc.tensor.matmul(out=pvb[:], lhsT=vts[:], rhs=idn[:GB, :GB],
                             start=True, stop=True)
            vt_g = wk.tile([D, GB], BF16, tag="vt")
            nc.vector.tensor_copy(out=vt_g[:], in_=pvb[:])

            # q = W_last^T hc + W_first^T hf + Wg' G + Wv V + W_state^T sf (+b)
            psq = ps_q.tile([D, GB], F32, tag="sm")
            nc.tensor.matmul(out=psq[:], lhsT=wcat[:, 0:D],
                             rhs=hct[:, GB * g:GB * (g + 1)], start=True, stop=True)
            nc.tensor.matmul(out=psq[:], lhsT=wcat[:, D:2 * D],
                             rhs=hft[:, GB * g:GB * (g + 1)], start=False,
                             stop=True, skip_group_check=True)
            nc.tensor.matmul(out=psq[:], lhsT=wgs[:], rhs=gt_g[:], start=False,
                             stop=True, skip_group_check=True)
            nc.tensor.matmul(out=psq[:], lhsT=wcat[:, 3 * D:4 * D], rhs=vt_g[:],
                             start=False, stop=True, skip_group_check=True)
            nc.tensor.matmul(out=psq[:], lhsT=ws[:],
                             rhs=sft[:, GB * g:GB * (g + 1)], start=False,
                             stop=True, skip_group_check=True)
            qt_g = wk.tile([D, GB], BF16, tag="qt")
            nc.vector.tensor_scalar(out=qt_g[:], in0=psq[:], scalar1=bst[:, :1],
                                    scalar2=None, op0=Alu.add)

            # qk = W_key^T q / sqrt(D)
            psk = ps_q.tile([D, GB], F32, tag="sm")
            nc.tensor.matmul(out=psk[:], lhsT=wcat[:, 4 * D:5 * D], rhs=qt_g[:],
                             start=True, stop=True)
            qk_g = wk.tile([D, GB], BF16, tag="qkg")
            nc.vector.tensor_scalar_mul(out=qk_g[:], in0=psk[:],
                                        scalar1=INV_SQRT_D)

            # scatter into qkp: batch b=16g+j -> col 32b + (b%32)
            base = 512 * g + 16 * (g % 2)
            nc.vector.tensor_copy(out=qkp[:, base:base + 33 * (GB - 1) + 1:33],
                                  in_=qk_g[:])

            # scores for this group
            J = g // 2
            for j in range(GB):
                b = GB * g + j
                nc.tensor.matmul(
                    out=pssc[32 * J:32 * J + 32, :],
                    lhsT=qkp[:, 32 * b:32 * b + 32],
                    rhs=et[g][:, N * j:N * (j + 1)],
                    start=(b % 32 == 0), stop=(b % 32 == 31),
                    tile_position=(0, 32 * J), skip_group_check=True)

            if g == 2:
                # travel select-accumulate (gpsimd gathers done by now)
                for k in range(16):
                    nc.tensor.matmul(out=ptv[:], lhsT=dkc[:, 128 * k:128 * (k + 1)],
                                     rhs=gk_all[:, N * k:N * (k + 1)],
                                     start=(k == 0), stop=(k == 15),
                                     skip_group_check=True)
                travf = cp.tile([BC, N], F32, name="travf")
                nc.vector.tensor_copy(out=travf[:], in_=ptv[:])

        # ---- epilogue: -travel, tanh-clip, mask, log_softmax ----
        if DEBUG_TAPS:
            for nm, tl in [("d_hct", hct), ("d_hft", hft), ("d_qkp", qkp),
                           ("d_trav", travf)]:
                tmpd = wk.tile([tl.shape[0], tl.shape[1]], F32, tag=f"tap{nm}")
                nc.vector.tensor_copy(out=tmpd[:], in_=tl[:])
                nc.sync.dma_start(out=ap[nm], in_=tmpd[:])
        ssb = wk.tile([BC, N], F32, tag="ssb")
        nc.vector.tensor_add(out=ssb[:], in0=pssc[:], in1=travf[:])
        if DEBUG_TAPS:
            nc.sync.dma_start(out=ap["d_score"], in_=ssb[:])
        th = wk.tile([BC, N], F32, tag="th")
        nc.scalar.activation(out=th[:], in_=ssb[:], func=AF.Tanh,
                             scale=1.0 / TANH_CLIP)
        msk = wk.tile([BC, N], F32, tag="msk")
        nc.vector.tensor_mul(out=msk[:], in0=th[:], in1=m10[:])
        nc.vector.tensor_add(out=msk[:], in0=msk[:], in1=m2[:])

        mx = cp.tile([BC, 1], F32, name="mx")
        nc.vector.tensor_reduce(out=mx[:], in_=msk[:], axis=AX.X, op=Alu.max)
        ngm = cp.tile([BC, 1], F32, name="ngm")
        nc.vector.tensor_scalar_mul(out=ngm[:], in0=mx[:], scalar1=-1.0)
        ex = wk.tile([BC, N], F32, tag="ex")
        se = cp.tile([BC, 1], F32, name="se")
        nc.scalar.activation(out=ex[:], in_=msk[:], func=AF.Exp, bias=ngm[:, :1],
                             scale=1.0, accum_out=se[:])
        lse = cp.tile([BC, 1], F32, name="lse")
        nc.scalar.activation(out=lse[:], in_=se[:], func=AF.Ln)
        fin = wk.tile([BC, N], F32, tag="fin")
        nc.vector.tensor_scalar(out=fin[:], in0=msk[:], scalar1=mx[:, :1],
                                scalar2=lse[:, :1], op0=Alu.subtract,
                                op1=Alu.subtract)
        nc.sync.dma_start(out=ap["out"], in_=fin[:])


def build_program():
    nc = bacc.Bacc("TRN2", target_bir_lowering=False, debug=False)
    dt = nc.dram_tensor
    T = {}

    def din(name, shape, dtype):
        T[name] = dt(name, shape, dtype, kind="ExternalInput")

    din("nat_f8", [NG, 128, GB * 4 * D], F8)
    din("et_bf", [NG, 128, GB * N], BF16)
    din("emb_flat", [BC * N, D], BF16)
    din("h3_flat", [BC * N, 1], I32)
    din("h3w", [128, 16 * 32], U16)
    din("ttm_bf", [N, N], BF16)
    din("vfc", [128, NG * 128], F8)
    din("wcat", [D, 5 * D], BF16)
    din("ws", [3, D], BF16)
    din("bst", [D, 1], F32)
    din("sc8", [BC, 8], F32)
    din("visam", [BC, 2 * N], U8)
    din("iota", [BC, 1], F32)
    din("ident", [128, 128], BF16)
    din("sselc", [128, 16 * 128], BF16)
    din("dkc", [128, 16 * 128], BF16)
    T["out"] = dt("out", [BC, N], F32, kind="ExternalOutput")
    if DEBUG_TAPS:
        for nm, shp in [("d_hct", [D, BC]), ("d_hft", [D, BC]),
                        ("d_qkp", [128, 32 * BC]), ("d_trav", [BC, N]),
                        ("d_score", [BC, N])]:
            T[nm] = dt(nm, shp, F32, kind="ExternalOutput")

    with tile.TileContext(nc) as tc:
        _emit(nc, tc, T)
    nc.compile()
    return nc


@functools.cache
def _cached_program():
    return build_program()


@functools.cache
def _consts():
    c = {}
    c["ident"] = np.eye(128, dtype=NBF)
    s = np.zeros((16, 128, 128), dtype=np.float32)
    dk = np.zeros((16, 128, 128), dtype=np.float32)
    pidx = np.arange(128)
    for k in range(16):
        s[k, (pidx // 16) * 16 + k, pidx] = C_TRAVEL
        rows = pidx[pidx % 16 == k]
        dk[k, rows, rows] = -1.0
    c["sselc"] = np.ascontiguousarray(s.transpose(1, 0, 2)).reshape(128, 2048).astype(NBF)
    c["dkc"] = np.ascontiguousarray(dk.transpose(1, 0, 2)).reshape(128, 2048).astype(NBF)
    c["iota"] = (np.arange(BC, dtype=np.float32) * N)[:, None]
    return c


def make_in_map(inputs, core, consts=None):
    """Host-side shard + relayout for one core (pure layout/dtype work)."""
    sl = slice(BC * core, BC * (core + 1))
    emb = np.asarray(inputs["node_emb"][sl], dtype=np.float32)
    embb = emb.astype(NBF)          # [128, 512, 128]
    embf8 = emb.astype(NF8)
    m = {}
    m["nat_f8"] = np.ascontiguousarray(
        embf8.reshape(NG, GB, 4, 128, D).transpose(0, 3, 1, 2, 4)
    ).reshape(NG, 128, GB * 4 * D)
    m["et_bf"] = np.ascontiguousarray(
        embb.transpose(0, 2, 1).reshape(NG, GB, D, N).transpose(0, 2, 1, 3)
    ).reshape(NG, 128, GB * N)
    m["emb_flat"] = embb.reshape(BC * N, D)
    h3 = np.asarray(inputs["h3_indices"][sl]).astype(np.int32)   # [128, 512]
    m["h3_flat"] = h3.reshape(BC * N, 1)
    m["h3w"] = np.ascontiguousarray(
        h3.reshape(8, 16, 32, 16).transpose(1, 0, 3, 2).reshape(16, 128, 32)
        .transpose(1, 0, 2)).reshape(128, 512).astype(np.uint16)
    m["ttm_bf"] = np.asarray(inputs["travel_time_matrix"], np.float32).astype(NBF)
    vis = np.asarray(inputs["visited"][sl]).astype(np.uint8)
    am = np.asarray(inputs["action_mask"][sl]).astype(np.uint8)
    m["visam"] = np.ascontiguousarray(np.concatenate([vis, am], axis=1))
    v = np.zeros((128, NG, 4, GB, 2), dtype=np.float32)
    v[:, :, :, :, 0] = 0.25
    v[:, :, :, :, 1] = 0.25 * vis.reshape(NG, GB, 4, 128).transpose(3, 0, 2, 1)
    m["vfc"] = np.ascontiguousarray(v).reshape(128, NG * 128).astype(NF8)
    wl = np.asarray(inputs["W_last"], np.float32)
    wf = np.asarray(inputs["W_first"], np.float32)
    wg = np.asarray(inputs["W_graph"], np.float32)
    wv = np.asarray(inputs["W_visited"], np.float32)
    wkT = np.asarray(inputs["W_key"], np.float32).T
    m["wcat"] = np.ascontiguousarray(
        np.concatenate([wl, wf, wg, wv, wkT], axis=1)).astype(NBF)
    m["ws"] = np.asarray(inputs["W_state"], np.float32).astype(NBF)
    m["bst"] = np.asarray(inputs["b_state"], np.float32).reshape(D, 1)
    m["sc8"] = np.ascontiguousarray(np.concatenate(
        [np.asarray(inputs["current_time"][sl], np.float32),
         np.asarray(inputs["used_capacity"][sl], np.float32),
         np.asarray(inputs["vehicle_capacity"][sl], np.float32),
         np.asarray(inputs["i"][sl]).astype(np.float32),
         np.asarray(inputs["current_node"][sl]).astype(np.float32),
         np.asarray(inputs["previous_action"][sl]).astype(np.float32),
         np.asarray(inputs["first_node"][sl]).astype(np.float32).reshape(BC, 1),
         np.zeros((BC, 1), np.float32)], axis=1))
    m.update(consts or _consts())
    return m


_last_results = None


def kernel(**inputs):
    global _last_results
    nc = _cached_program()
    consts = _consts()
    in_maps = [make_in_map(inputs, c, consts) for c in range(NCORES)]
    import os
    trace = bool(int(os.environ.get("KERNEL_TRACE", "0")))
    rr = run_bass_kernel_spmd(nc, in_maps, list(range(NCORES)), trace=trace)
    _last_results = rr
    out = np.concatenate([np.asarray(rr.results[c]["out"], np.float32)
                          for c in range(NCORES)], axis=0)
    return out


# revision 21
# speedup vs baseline: 1.2085x; 1.0101x over previous
"""Trainium2 Bass kernel for nn_DARPDecoder (sparse_attention).

Strategy (pure data-parallel over batch, 8 cores x 128 batches), single-pass
per-group pipeline:
  score[b,n] = emb[b,n,:].qk[b]/sqrt(D) - travel[b,n]*c ; tanh-clip, mask,
  log_softmax, with qk[b] = W_key^T q[b].
Per core, TWO HBM streams consumed in 8 groups of 16 batches each:
  - nat_f8 (natural [n,d] tiles, fp8-e4m3, 8.4MB): graph/visited sums via
    "flipped" matmuls -- the nat tile is the STATIONARY, the moving operand is
    a tiny [128,2] (const 0.25 | 0.25*vf) column pair, accumulating [128d, 2]
    per batch into a per-group PSUM [128, 32].  fp8 is plenty for sums.
  - et_bf (transposed [d,n] tiles, bf16, 16.8MB): per-batch score matmuls with
    zero-padded qk columns accumulate into one [128b, 512n] PSUM.
Each group's sums -> qk -> scores chases its chunk DMAs, so everything hides
behind the ~70us of HBM traffic.  Total DMA count is ~30 (vs 200+): travel
rows are extracted from the gpsimd gather output with 16 selection matmuls
accumulating into a PSUM travel tile instead of 128 tiny row DMAs.
Gathers: indirect DMA rows (h_cur/h_first from bf16 emb_flat, T rows by
cur_h3); travel lookup T[cur_h3[b], h3[b,n]] via gpsimd indirect_copy on
16-partition-replicated rows (host pre-wrapped h3 index layout).
Per-batch 1/vcount descale of the visited sum is applied exactly via a
transpose sandwich: V_raw^T -> [16b,128d] (batch on partitions) -> per-
partition tensor_scalar multiply -> transpose back.
"""

import functools
import math

import numpy as np
import ml_dtypes

import concourse.bass as bass
import concourse.mybir as mybir
import concourse.tile as tile
from concourse import bacc
from concourse.bass_utils import run_bass_kernel_spmd

BF16 = mybir.dt.bfloat16
F8 = mybir.dt.float8e4
F32 = mybir.dt.float32
I32 = mybir.dt.int32
U16 = mybir.dt.uint16
U8 = mybir.dt.uint8
Alu = mybir.AluOpType
AF = mybir.ActivationFunctionType
AX = mybir.AxisListType

B, N, D, NCORES = 1024, 512, 128, 8
BC = B // NCORES   # 128 batches/core
NG, GB = 8, 16     # 8 groups x 16 batches
MAX_TIME = 1440.0
TANH_CLIP = 10.0
C_TRAVEL = 1.0 / MAX_TIME / math.sqrt(2.0)
INV_SQRT_D = 1.0 / math.sqrt(D)
NBF = np.dtype(ml_dtypes.bfloat16)
NF8 = np.dtype(ml_dtypes.float8_e4m3)
BLOB_BYTES = 13096
DEBUG_TAPS = False


def _emit(nc, tc, T):
    ap = {k: v.ap() for k, v in T.items()}

    with (
        tc.tile_pool(name="cp", bufs=1) as cp,
        tc.tile_pool(name="wk", bufs=2) as wk,
        tc.tile_pool(name="stn", bufs=6) as stn,
        tc.tile_pool(name="ste", bufs=10) as ste,
        tc.tile_pool(name="ps_sc", bufs=1, space="PSUM") as ps_sc,
        tc.tile_pool(name="ps_rep", bufs=2, space="PSUM") as ps_rep,
        tc.tile_pool(name="ps_sm", bufs=2, space="PSUM") as ps_sm,
        tc.tile_pool(name="ps_q", bufs=2, space="PSUM") as ps_q,
    ):
        # ---- DMA issue: one consts blob, then the two streams, all on SP
        #      (one HWDGE slot per big transfer; never issue-rate bound) ----
        blob = cp.tile([128, BLOB_BYTES], U8, name="blob")
        nc.sync.dma_start(out=blob[:], in_=ap["blob"])
        sc8 = blob[:, 0:32].bitcast(F32)
        iota = blob[:, 32:36].bitcast(F32)
        bst = blob[:, 36:40].bitcast(F32)
        vfc = blob[:, 40:1064].bitcast(F8)
        visam = blob[:, 1064:2088]
        wcat = blob[:, 2088:3368].bitcast(BF16)
        ws = blob[:3, 3368:3624].bitcast(BF16)
        idn = blob[:, 3624:3880].bitcast(BF16)
        h3w = blob[:, 3880:4904].bitcast(U16)
        sselc = blob[:, 4904:9000].bitcast(BF16)
        dkc = blob[:, 9000:13096].bitcast(BF16)

        nat = [None] * NG
        for g in range(NG):
            nat[g] = stn.tile([128, GB * 4 * D], F8, tag="nat", name=f"nat{g}")
            nc.sync.dma_start(out=nat[g][:], in_=ap["nat_f8"][g])
        et = [None] * 16
        for h in range(16):
            et[h] = ste.tile([128, 8 * N], BF16, tag="et", name=f"et{h}")
            nc.sync.dma_start(out=et[h][:], in_=ap["et_bf"][h])

        # zero stationary for the pssc-clearing matmul
        zc = cp.tile([128, 128], BF16, name="zc")
        nc.vector.memset(zc[:], 0.0)

        # warm the ACT table set (copy/exp/ln all live in one set)
        actw = cp.tile([1, 1], F32, name="actw")
        nc.vector.memset(actw[:], 0.0)
        nc.scalar.activation(out=actw[:], in_=actw[:], func=AF.Exp)

        # ---- qkp (zero-padded qk columns), memset early: zero deps ----
        qkp = cp.tile([128, 32 * BC], BF16, name="qkp")
        nc.vector.memset(qkp[:], 0.0)

        # ---- scalar state / first-node bookkeeping (DVE, f32 exact) ----
        t1 = cp.tile([BC, 1], F32, name="t1")
        nc.vector.tensor_single_scalar(out=t1[:], in_=sc8[:, 5:6], scalar=0.0,
                                       op=Alu.is_equal)
        t2 = cp.tile([BC, 1], F32, name="t2")
        nc.vector.tensor_single_scalar(out=t2[:], in_=sc8[:, 4:5], scalar=0.0,
                                       op=Alu.not_equal)
        ld = cp.tile([BC, 1], F32, name="ld")
        nc.vector.tensor_mul(out=ld[:], in0=t1[:], in1=t2[:])
        dd = cp.tile([BC, 1], F32, name="dd")
        nc.vector.tensor_sub(out=dd[:], in0=sc8[:, 4:5], in1=sc8[:, 6:7])
        nc.vector.tensor_mul(out=dd[:], in0=ld[:], in1=dd[:])
        fnf = cp.tile([BC, 1], F32, name="fnf")
        nc.vector.tensor_add(out=fnf[:], in0=sc8[:, 6:7], in1=dd[:])
        nc.vector.tensor_mul(out=fnf[:], in0=fnf[:], in1=t2[:])

        gcf = cp.tile([BC, 1], F32, name="gcf")
        nc.vector.tensor_add(out=gcf[:], in0=iota, in1=sc8[:, 4:5])
        gcur = cp.tile([BC, 1], I32, name="gcur")
        nc.vector.tensor_copy(out=gcur[:], in_=gcf[:])
        gff = cp.tile([BC, 1], F32, name="gff")
        nc.vector.tensor_add(out=gff[:], in0=iota, in1=fnf[:])
        gfn = cp.tile([BC, 1], I32, name="gfn")
        nc.vector.tensor_copy(out=gfn[:], in_=gff[:])

        # visited count -> vcr4 = 4/max(vc,1) (bf16 col for ident matmul rhs)
        visf = cp.tile([BC, N], F32, name="visf")
        nc.vector.tensor_copy(out=visf[:], in_=visam[:, :N])
        vc = cp.tile([BC, 1], F32, name="vc")
        nc.vector.tensor_reduce(out=vc[:], in_=visf[:], axis=AX.X, op=Alu.add)
        nc.vector.tensor_scalar_max(out=vc[:], in0=vc[:], scalar1=1.0)
        nc.vector.tensor_scalar_mul(out=vc[:], in0=vc[:], scalar1=0.25)
        vcr4 = cp.tile([BC, 1], F32, name="vcr4")
        nc.vector.reciprocal(out=vcr4[:], in_=vc[:])
        vcr4b = cp.tile([BC, 1], BF16, name="vcr4b")
        nc.vector.tensor_copy(out=vcr4b[:], in_=vcr4[:])

        # action mask precompute
        amf = cp.tile([BC, N], F32, name="amf")
        nc.vector.tensor_copy(out=amf[:], in_=visam[:, N:])
        mA = cp.tile([BC, N], F32, name="mA")
        nc.vector.tensor_scalar(out=mA[:], in0=amf[:], scalar1=1.0, scalar2=1e8,
                                op0=Alu.subtract, op1=Alu.mult)
        m10 = cp.tile([BC, N], F32, name="m10")
        nc.vector.tensor_scalar_mul(out=m10[:], in0=amf[:], scalar1=TANH_CLIP)
        nc.vector.tensor_add(out=mA[:], in0=mA[:], in1=m10[:])
        mB = cp.tile([BC, N], F32, name="mB")
        nc.vector.tensor_scalar_mul(out=mB[:], in0=amf[:], scalar1=2.0 * TANH_CLIP)

        # state feats [BC,3] -> sft [3,BC]
        sfb = cp.tile([BC, 3], BF16, name="sfb")
        nc.vector.tensor_sub(out=sfb[:, 0:1], in0=sc8[:, 2:3], in1=sc8[:, 1:2])
        nc.vector.tensor_scalar_mul(out=sfb[:, 1:2], in0=sc8[:, 0:1],
                                    scalar1=1.0 / MAX_TIME)
        nc.vector.tensor_scalar_mul(out=sfb[:, 2:3], in0=sc8[:, 3:4],
                                    scalar1=1.0 / (2.0 * N))
        psf = ps_q.tile([128, 128], BF16, tag="sm")
        nc.tensor.transpose(out=psf[:3, :], in_=sfb[:], identity=idn)
        sft = cp.tile([3, BC], BF16, name="sft")
        nc.vector.tensor_copy(out=sft[:], in_=psf[:3, :BC])

        # wg scaled by 1/128 (sums use 0.25 weights; graph mean needs /512)
        wgs = cp.tile([D, D], BF16, name="wgs")
        nc.vector.tensor_scalar_mul(out=wgs[:], in0=wcat[:, 2 * D:3 * D],
                                    scalar1=1.0 / 128.0)

        # ---- gathers (Pool/SWDGE queue) ----
        hc_rows = cp.tile([BC, D], BF16, name="hc_rows")
        nc.gpsimd.indirect_dma_start(
            out=hc_rows, out_offset=None, in_=ap["emb_flat"],
            in_offset=bass.IndirectOffsetOnAxis(ap=gcur[:, :1], axis=0))
        hf_rows = cp.tile([BC, D], BF16, name="hf_rows")
        nc.gpsimd.indirect_dma_start(
            out=hf_rows, out_offset=None, in_=ap["emb_flat"],
            in_offset=bass.IndirectOffsetOnAxis(ap=gfn[:, :1], axis=0))
        ch3 = cp.tile([BC, 1], I32, name="ch3")
        nc.gpsimd.indirect_dma_start(
            out=ch3[:], out_offset=None, in_=ap["h3_flat"],
            in_offset=bass.IndirectOffsetOnAxis(ap=gcur[:, :1], axis=0))
        rrow = cp.tile([BC, N], BF16, name="rrow")
        nc.gpsimd.indirect_dma_start(
            out=rrow[:], out_offset=None, in_=ap["ttm_bf"],
            in_offset=bass.IndirectOffsetOnAxis(ap=ch3[:, :1], axis=0))

        # ---- h_cur / h_first transposes -> [128d, BC] bf16 ----
        hct = cp.tile([D, BC], BF16, name="hct")
        pt1 = ps_q.tile([128, 128], BF16, tag="sm")
        nc.tensor.transpose(out=pt1[:], in_=hc_rows, identity=idn)
        nc.vector.tensor_copy(out=hct[:], in_=pt1[:])
        hft = cp.tile([D, BC], BF16, name="hft")
        pt2 = ps_q.tile([128, 128], BF16, tag="sm")
        nc.tensor.transpose(out=pt2[:], in_=hf_rows, identity=idn)
        nc.vector.tensor_copy(out=hft[:], in_=pt2[:])

        gk_all = cp.tile([128, 16 * N], BF16, name="gk_all")
        pssc = ps_sc.tile([128, N], F32, tag="score")
        # clear pssc once; every later matmul (travel + scores) accumulates
        nc.tensor.matmul(out=pssc[:], lhsT=zc[:], rhs=qkp[:, :N], start=True,
                         stop=False, skip_group_check=True)

        # ---- loop A: per-group sums -> qk (chases the nat stream); travel
        #      replication/gather interleaved 2 calls per group ----
        for g in range(NG):
            # sums: flipped matmuls, nat tile slices as stationaries
            psS = ps_sm.tile([128, 2 * GB], F32, tag="sums")
            for j in range(GB):
                for t in range(4):
                    nc.tensor.matmul(
                        out=psS[:, 2 * j:2 * j + 2],
                        lhsT=nat[g][:, (j * 4 + t) * D:(j * 4 + t + 1) * D],
                        rhs=vfc[:, 128 * g + 32 * t + 2 * j:
                                 128 * g + 32 * t + 2 * j + 2],
                        start=(t == 0), stop=(t == 3), skip_group_check=True)

            # graph cols (even) / raw visited cols (odd) -> SBUF bf16
            gt_g = wk.tile([D, GB], BF16, tag="gt")
            nc.vector.tensor_copy(
                out=gt_g[:], in_=psS[:].rearrange("p (s c) -> p s c", c=2)[:, :, 0])
            vr_g = wk.tile([D, GB], BF16, tag="vr")
            nc.vector.tensor_copy(
                out=vr_g[:], in_=psS[:].rearrange("p (s c) -> p s c", c=2)[:, :, 1])

            # 1/vcount descale sandwich: transpose, per-partition scale, back
            vcg = ps_q.tile([GB, 1], F32, tag="sm")
            nc.tensor.matmul(out=vcg[:], lhsT=idn[:, GB * g:GB * (g + 1)],
                             rhs=vcr4b[:], start=True, stop=True)
            vcgs = wk.tile([GB, 1], F32, tag="vcgs")
            nc.vector.tensor_copy(out=vcgs[:], in_=vcg[:])
            pvt = ps_q.tile([GB, D], BF16, tag="sm")
            nc.tensor.transpose(out=pvt[:], in_=vr_g[:], identity=idn)
            vts = wk.tile([GB, D], BF16, tag="vts")
            nc.vector.tensor_scalar(out=vts[:], in0=pvt[:], scalar1=vcgs[:, :1],
                                    scalar2=None, op0=Alu.mult)
            pvb = ps_q.tile([D, GB], F32, tag="sm")
            nc.tensor.matmul(out=pvb[:], lhsT=vts[:], rhs=idn[:GB, :GB],
                             start=True, stop=True)
            vt_g = wk.tile([D, GB], BF16, tag="vt")
            nc.vector.tensor_copy(out=vt_g[:], in_=pvb[:])

            # q = W_last^T hc + W_first^T hf + Wg' G + Wv V + W_state^T sf (+b)
            psq = ps_q.tile([D, GB], F32, tag="sm")
            nc.tensor.matmul(out=psq[:], lhsT=wcat[:, 0:D],
                             rhs=hct[:, GB * g:GB * (g + 1)], start=True, stop=True)
            nc.tensor.matmul(out=psq[:], lhsT=wcat[:, D:2 * D],
                             rhs=hft[:, GB * g:GB * (g + 1)], start=False,
                             stop=True, skip_group_check=True)
            nc.tensor.matmul(out=psq[:], lhsT=wgs[:], rhs=gt_g[:], start=False,
                             stop=True, skip_group_check=True)
            nc.tensor.matmul(out=psq[:], lhsT=wcat[:, 3 * D:4 * D], rhs=vt_g[:],
                             start=False, stop=True, skip_group_check=True)
            nc.tensor.matmul(out=psq[:], lhsT=ws,
                             rhs=sft[:, GB * g:GB * (g + 1)], start=False,
                             stop=True, skip_group_check=True)
            qt_g = wk.tile([D, GB], BF16, tag="qt")
            nc.vector.tensor_scalar(out=qt_g[:], in0=psq[:], scalar1=bst[:, :1],
                                    scalar2=None, op0=Alu.add)

            # qk = W_key^T q / sqrt(D)
            psk = ps_q.tile([D, GB], F32, tag="sm")
            nc.tensor.matmul(out=psk[:], lhsT=wcat[:, 4 * D:5 * D], rhs=qt_g[:],
                             start=True, stop=True)
            qk_g = wk.tile([D, GB], BF16, tag="qkg")
            nc.vector.tensor_scalar_mul(out=qk_g[:], in0=psk[:],
                                        scalar1=INV_SQRT_D)

            # scatter into qkp: batch b=16g+j -> col 32b + (b%32)
            base = 512 * g + 16 * (g % 2)
            nc.vector.tensor_copy(out=qkp[:, base:base + 33 * (GB - 1) + 1:33],
                                  in_=qk_g[:])

            # travel replication + gpsimd gather, 2 calls per group
            for k in (2 * g, 2 * g + 1):
                prep = ps_rep.tile([128, N], F32, tag="rep")
                nc.tensor.matmul(out=prep[:], lhsT=sselc[:, 128 * k:128 * (k + 1)],
                                 rhs=rrow[:], start=True, stop=True)
                sck = wk.tile([128, N], BF16, tag="sck")
                nc.scalar.activation(out=sck[:], in_=prep[:], func=AF.Copy)
                nc.gpsimd.indirect_copy(out=gk_all[:, N * k:N * (k + 1)],
                                        data=sck[:],
                                        idxs=h3w[:, 32 * k:32 * (k + 1)],
                                        i_know_ap_gather_is_preferred=True)

        # ---- travel select-accumulate straight into the score PSUM ----
        for k in range(16):
            nc.tensor.matmul(out=pssc[:], lhsT=dkc[:, 128 * k:128 * (k + 1)],
                             rhs=gk_all[:, N * k:N * (k + 1)],
                             start=False, stop=False, skip_group_check=True)

        # ---- loop B: per-half-group scores (chases the et stream), with the
        #      epilogue + output DMA emitted per 32-row band as it completes ----
        th = cp.tile([BC, N], F32, name="th")
        msk = cp.tile([BC, N], F32, name="msk")
        ex = cp.tile([BC, N], F32, name="ex")
        fin = cp.tile([BC, N], F32, name="fin")
        se = cp.tile([BC, 1], F32, name="se")
        lse = cp.tile([BC, 1], F32, name="lse")
        for h in range(16):
            J = h // 4
            for j in range(8):
                b = 8 * h + j
                nc.tensor.matmul(
                    out=pssc[32 * J:32 * J + 32, :],
                    lhsT=qkp[:, 32 * b:32 * b + 32],
                    rhs=et[h][:, N * j:N * (j + 1)],
                    start=False, stop=(b % 32 == 31),
                    tile_position=(0, 32 * J), skip_group_check=True)
            if h % 4 == 3:
                sl = slice(32 * J, 32 * J + 32)
                # 10*tanh(x/10) = 10 - 20/(exp(x/5)+1): stays in the exp/ln
                # act-table set (no per-band table reloads).  Post-tanh scores
                # are clipped to [-10,10], so log-sum-exp needs no max shift.
                nc.scalar.activation(out=th[sl], in_=pssc[sl], func=AF.Exp,
                                     scale=2.0 / TANH_CLIP)
                nc.vector.tensor_scalar_add(out=th[sl], in0=th[sl], scalar1=1.0)
                nc.vector.reciprocal(out=th[sl], in_=th[sl])
                nc.vector.tensor_mul(out=th[sl], in0=th[sl], in1=mB[sl])
                nc.vector.tensor_sub(out=msk[sl], in0=mA[sl], in1=th[sl])
                nc.scalar.activation(out=ex[sl], in_=msk[sl], func=AF.Exp,
                                     scale=1.0, accum_out=se[sl])
                nc.scalar.activation(out=lse[sl], in_=se[sl], func=AF.Ln)
                nc.vector.tensor_scalar(out=fin[sl], in0=msk[sl],
                                        scalar1=lse[sl.start:sl.stop, :1],
                                        scalar2=None, op0=Alu.subtract)
                nc.gpsimd.dma_start(out=ap["out"][sl], in_=fin[sl])
                if J < 3:
                    # re-pin the exp table so the next band's Exp needs no
                    # act-table reload (Ln lives in a different set)
                    nc.scalar.activation(out=actw[:], in_=actw[:], func=AF.Exp)



def build_program():
    nc = bacc.Bacc("TRN2", target_bir_lowering=False, debug=False)
    dt = nc.dram_tensor
    T = {}

    def din(name, shape, dtype):
        T[name] = dt(name, shape, dtype, kind="ExternalInput")

    din("nat_f8", [NG, 128, GB * 4 * D], F8)
    din("et_bf", [16, 128, 8 * N], BF16)
    din("emb_flat", [BC * N, D], BF16)
    din("h3_flat", [BC * N, 1], I32)
    din("ttm_bf", [N, N], BF16)
    din("blob", [128, BLOB_BYTES], U8)
    T["out"] = dt("out", [BC, N], F32, kind="ExternalOutput")
    if DEBUG_TAPS:
        for nm, shp in [("d_hct", [D, BC]), ("d_hft", [D, BC]),
                        ("d_qkp", [128, 32 * BC]), ("d_trav", [BC, N]),
                        ("d_score", [BC, N])]:
            T[nm] = dt(nm, shp, F32, kind="ExternalOutput")

    with tile.TileContext(nc) as tc:
        _emit(nc, tc, T)
    nc.compile()
    return nc


@functools.cache
def _cached_program():
    return build_program()


@functools.cache
def _consts():
    c = {}
    c["ident"] = np.eye(128, dtype=NBF)
    s = np.zeros((16, 128, 128), dtype=np.float32)
    dk = np.zeros((16, 128, 128), dtype=np.float32)
    pidx = np.arange(128)
    for k in range(16):
        s[k, (pidx // 16) * 16 + k, pidx] = C_TRAVEL
        rows = pidx[pidx % 16 == k]
        dk[k, rows, rows] = -1.0
    c["sselc"] = np.ascontiguousarray(s.transpose(1, 0, 2)).reshape(128, 2048).astype(NBF)
    c["dkc"] = np.ascontiguousarray(dk.transpose(1, 0, 2)).reshape(128, 2048).astype(NBF)
    c["iota"] = (np.arange(BC, dtype=np.float32) * N)[:, None]
    return c


def make_in_map(inputs, core, consts=None):
    """Host-side shard + relayout for one core (pure layout/dtype work)."""
    sl = slice(BC * core, BC * (core + 1))
    emb = np.asarray(inputs["node_emb"][sl], dtype=np.float32)
    embb = emb.astype(NBF)          # [128, 512, 128]
    embf8 = emb.astype(NF8)
    m = {}
    m["nat_f8"] = np.ascontiguousarray(
        embf8.reshape(NG, GB, 4, 128, D).transpose(0, 3, 1, 2, 4)
    ).reshape(NG, 128, GB * 4 * D)
    m["et_bf"] = np.ascontiguousarray(
        embb.transpose(0, 2, 1).reshape(16, 8, D, N).transpose(0, 2, 1, 3)
    ).reshape(16, 128, 8 * N)
    m["emb_flat"] = embb.reshape(BC * N, D)
    h3 = np.asarray(inputs["h3_indices"][sl]).astype(np.int32)   # [128, 512]
    m["h3_flat"] = h3.reshape(BC * N, 1)
    h3w = np.ascontiguousarray(
        h3.reshape(8, 16, 32, 16).transpose(1, 0, 3, 2).reshape(16, 128, 32)
        .transpose(1, 0, 2)).reshape(128, 512).astype(np.uint16)
    m["ttm_bf"] = np.asarray(inputs["travel_time_matrix"], np.float32).astype(NBF)
    vis = np.asarray(inputs["visited"][sl]).astype(np.uint8)
    am = np.asarray(inputs["action_mask"][sl]).astype(np.uint8)
    visam = np.ascontiguousarray(np.concatenate([vis, am], axis=1))
    v = np.zeros((128, NG, 4, GB, 2), dtype=np.float32)
    v[:, :, :, :, 0] = 0.25
    v[:, :, :, :, 1] = 0.25 * vis.reshape(NG, GB, 4, 128).transpose(3, 0, 2, 1)
    vfc = np.ascontiguousarray(v).reshape(128, NG * 128).astype(NF8)
    wl = np.asarray(inputs["W_last"], np.float32)
    wf = np.asarray(inputs["W_first"], np.float32)
    wg = np.asarray(inputs["W_graph"], np.float32)
    wv = np.asarray(inputs["W_visited"], np.float32)
    wkT = np.asarray(inputs["W_key"], np.float32).T
    wcat = np.ascontiguousarray(
        np.concatenate([wl, wf, wg, wv, wkT], axis=1)).astype(NBF)
    wsp = np.zeros((128, 128), dtype=NBF)
    wsp[:3] = np.asarray(inputs["W_state"], np.float32).astype(NBF)
    bst = np.asarray(inputs["b_state"], np.float32).reshape(D, 1)
    sc8 = np.ascontiguousarray(np.concatenate(
        [np.asarray(inputs["current_time"][sl], np.float32),
         np.asarray(inputs["used_capacity"][sl], np.float32),
         np.asarray(inputs["vehicle_capacity"][sl], np.float32),
         np.asarray(inputs["i"][sl]).astype(np.float32),
         np.asarray(inputs["current_node"][sl]).astype(np.float32),
         np.asarray(inputs["previous_action"][sl]).astype(np.float32),
         np.asarray(inputs["first_node"][sl]).astype(np.float32).reshape(BC, 1),
         np.zeros((BC, 1), np.float32)], axis=1))
    c = consts or _consts()
    u8 = np.uint8
    m["blob"] = np.ascontiguousarray(np.concatenate([
        sc8.view(u8), c["iota"].view(u8), bst.view(u8), vfc.view(u8),
        visam, wcat.view(u8), wsp[:, :128].view(u8), c["ident"].view(u8),
        h3w.view(u8), c["sselc"].view(u8), c["dkc"].view(u8)], axis=1))
    assert m["blob"].shape == (128, BLOB_BYTES), m["blob"].shape
    return m


_last_results = None


def kernel(**inputs):
    global _last_results
    nc = _cached_program()
    consts = _consts()
    in_maps = [make_in_map(inputs, c, consts) for c in range(NCORES)]
    import os
    trace = bool(int(os.environ.get("KERNEL_TRACE", "0")))
    rr = run_bass_kernel_spmd(nc, in_maps, list(range(NCORES)), trace=trace)
    _last_results = rr
    out = np.concatenate([np.asarray(rr.results[c]["out"], np.float32)
                          for c in range(NCORES)], axis=0)
    return out
